# revision 41
# baseline (speedup 1.0000x reference)
# Additive attention kernel for Trainium2, SPMD over 8 NeuronCores.
#
# Reference computation (L=2048, B=32, Qd=Td=1024):
#   pre[l,b,d]  = sum_c T[l,b,c]*W1t[d,c] + (q[b] @ W1q.T + b1)[d]
#   poids       = tanh(pre)
#   s[l,b]      = sum_d poids[l,b,d] * v[d]
#   aw          = softmax_l(s)
#   attn[b,d]   = sum_l aw[l,b] * T[l,b,d]
#
# Sharding: data-parallel over batch. Each core gets 4 of 32 batch elements
# (full L), so softmax over L and the weighted sum need no collectives.
#
# Per-core dataflow (64 chunks of 128 (b,l)-rows, b-major):
#   DMA T tile [128n, 1024c] -> PE transpose -> X_T [128c, 8, 128n]
#   main matmul (fp32r): psum[n,d] += X_T(c)^T @ W1tT(c,d), 8 k-steps x 2 halves
#   + rank-1 matmul ones[1,128] (x) qb[b] adds the query/bias term
#   ACT tanh -> poids; DVE tensor_tensor_reduce(poids * v_bc) -> s column
#   ACT exp(s) -> e column (softmax without max subtraction: |s| <= sum|v| ~ 16,
#   exp stays well inside fp32 range; aw = e/Z is mathematically identical)
#   step5 matmul: A[b] += e_col^T @ T_tile, PSUM-accumulated over the 16
#   chunks of each batch stripe.
# End: PE-transpose e-columns to rows, row-sums -> Z, normalize aw and A on
# device, DMA out.

import numpy as np

L, B, QD, TD = 2048, 32, 1024, 1024
NCORES = 8
BSH = B // NCORES          # 4 batch elements per core
P = 128                    # partitions
NCH = (L * BSH) // P       # 64 chunks per core
CPB = L // P               # 16 chunks per batch element
KC = TD // P               # 8 contraction sub-blocks
H = TD // 2                # 512, half of the free dim (one PSUM bank)

_cache = {}


def _build(mm_f32r=True, tr_f32r=True, step5=True, epilogue=True, sact=True,
           use_ttr=False, qb_dve=True):
    # use_ttr=True (fused DVE tensor_tensor_reduce) crashes the exec unit on
    # this runtime (NRT_EXEC_UNIT_UNRECOVERABLE); the mul+reduce pair works.
    from contextlib import ExitStack

    import concourse.bass as bass
    import concourse.bacc as bacc_mod
    import concourse.tile as tile
    import concourse.mybir as mybir

    f32 = mybir.dt.float32
    f32r = mybir.dt.float32r
    ACT = mybir.ActivationFunctionType

    # dtype for tensors feeding the fp32r matmuls (must be materialized as
    # float32r for the BIR verifier; bit layout is identical to fp32)
    mmdt = f32r if mm_f32r else f32

    def dram_mm(ap):
        # DRAM-side AP view for DMA into an mmdt tile (exact bits)
        return ap.bitcast(f32r) if mm_f32r else ap

    def as_f32(ap):
        # view an mmdt tile as plain fp32 (for non-f32r consumers)
        return ap.bitcast(f32) if mm_f32r else ap

    nc = bacc_mod.Bacc("TRN2")

    t_d = nc.declare_dram_parameter("targets", [L, BSH, TD], f32, isOutput=False)
    w_d = nc.declare_dram_parameter("w", [TD, TD], f32, isOutput=False)  # W1t.T [c,d]
    qb_d = nc.declare_dram_parameter("qb", [BSH, TD], f32, isOutput=False)
    v_d = nc.declare_dram_parameter("v", [1, TD], f32, isOutput=False)
    id_d = nc.declare_dram_parameter("ident", [P, P], f32, isOutput=False)
    ones_d = nc.declare_dram_parameter("ones", [1, P], f32, isOutput=False)
    attn_d = nc.declare_dram_parameter("attn", [BSH, TD], f32, isOutput=True)
    aw_d = nc.declare_dram_parameter("aw", [BSH, L], f32, isOutput=True)

    with tile.TileContext(nc) as tc, ExitStack() as ctx:
        singles = ctx.enter_context(tc.tile_pool(name="singles", bufs=1))
        xpool = ctx.enter_context(tc.tile_pool(name="xnat", bufs=3))
        xtpool = ctx.enter_context(tc.tile_pool(name="xt", bufs=3))
        ppool = ctx.enter_context(tc.tile_pool(name="poids", bufs=3))
        spool = ctx.enter_context(tc.tile_pool(name="scratch", bufs=2))
        scpool = ctx.enter_context(tc.tile_pool(name="scol", bufs=3))
        prepool = ctx.enter_context(tc.tile_pool(name="pre", bufs=3))
        qbcpool = ctx.enter_context(tc.tile_pool(name="qbc", bufs=2))
        tppool = ctx.enter_context(tc.tile_pool(name="tpsum", bufs=2, space="PSUM"))
        mmpool = ctx.enter_context(tc.tile_pool(name="mmpsum", bufs=4, space="PSUM"))
        apool = ctx.enter_context(tc.tile_pool(name="apsum", bufs=1, space="PSUM"))

        # --- resident tiles ---
        # small setup tensors first on HWDGE so the first transposes/matmuls
        # aren't queued behind the 4MB W transfer; W itself goes over SWDGE
        # (gpsimd) so the HWDGE queues stay dedicated to the x_nat stream
        trdt = mmdt if tr_f32r else f32
        ident = singles.tile([P, P], trdt)
        nc.sync.dma_start(
            out=ident, in_=id_d[:, :].bitcast(trdt) if tr_f32r else id_d[:, :]
        )

        v_bc = singles.tile([P, TD], f32)
        nc.gpsimd.dma_start(out=v_bc, in_=v_d[0:1, :].to_broadcast([P, TD]))

        if not qb_dve:
            qb_sb = singles.tile([1, BSH, TD], mmdt)
            nc.sync.dma_start(out=qb_sb, in_=dram_mm(qb_d[:, :].unsqueeze(0)))
            ones_sb = singles.tile([1, P], mmdt)
            nc.sync.dma_start(out=ones_sb, in_=dram_mm(ones_d[:, :]))

        w_sb = singles.tile([P, KC, TD], mmdt)
        nc.gpsimd.dma_start(
            out=w_sb, in_=dram_mm(w_d.rearrange("(ci cp) d -> cp ci d", cp=P))
        )

        e_cols = singles.tile([P, NCH], mmdt)   # exp(s), one column per chunk
        a_sb = singles.tile([1, BSH, TD], f32)  # unnormalized attn rows

        # --- main loop ---
        # state carried between iterations for the software-pipelined step5
        prev = None  # (x_nat, k) of previous chunk
        a_ps = None
        qb_bc = None

        for k in range(NCH):
            b = k // CPB
            l0 = (k % CPB) * P

            if qb_dve and k % CPB == 0:
                # per-stripe broadcast of qb[b] across partitions (DVE adds it
                # to the matmul output instead of a PE rank-1 matmul)
                qb_bc = qbcpool.tile([P, TD], f32, tag="qbc")
                nc.sync.dma_start(
                    out=qb_bc, in_=qb_d[b : b + 1, :].to_broadcast([P, TD])
                )

            x_nat = xpool.tile([P, TD], mmdt, tag="xnat")
            nc.sync.dma_start(out=x_nat, in_=dram_mm(t_d[l0 : l0 + P, b, :]))

            # transpose x_nat -> x_t [c, ci, n] (plain fp32 transpose path).
            # Two separate tiles, one per copy engine, so each matmul has a
            # single producer to wait on (walrus limits sync waits per inst).
            x_t_lo = xtpool.tile([P, 4, P], mmdt, tag="xtlo")
            x_t_hi = xtpool.tile([P, 4, P], mmdt, tag="xthi")
            for h in range(2):
                tp = tppool.tile([P, 4, P], trdt, tag="tp")
                for j in range(4):
                    ci = 4 * h + j
                    src = x_nat[:, ci * P : (ci + 1) * P]
                    nc.tensor.transpose(
                        tp[:, j, :], src if tr_f32r else as_f32(src), ident
                    )
                if h == 0:
                    nc.vector.tensor_copy(out=x_t_lo, in_=tp)
                else:
                    nc.scalar.copy(out=x_t_hi, in_=tp)

            # main matmul: two independent single-bank half tiles so each
            # half's PSUM slot frees as soon as its own DVE add finishes
            ps0 = mmpool.tile([P, H], f32, tag="mm")
            ps1 = mmpool.tile([P, H], f32, tag="mm")
            for ci in range(KC):
                lhsT = x_t_lo[:, ci, :] if ci < 4 else x_t_hi[:, ci - 4, :]
                last_ci = qb_dve and ci == KC - 1
                nc.tensor.matmul(
                    ps0, lhsT, w_sb[:, ci, 0:H],
                    start=(ci == 0), stop=last_ci,
                )
                nc.tensor.matmul(
                    ps1, lhsT, w_sb[:, ci, H:TD],
                    start=(ci == 0), stop=last_ci,
                )
            if not qb_dve:
                # rank-1 bias: ones (x) qb[b]
                nc.tensor.matmul(
                    ps0, ones_sb, qb_sb[0:1, b, 0:H],
                    start=False, stop=True,
                )
                nc.tensor.matmul(
                    ps1, ones_sb, qb_sb[0:1, b, H:TD],
                    start=False, stop=True,
                )

            # step5 for the PREVIOUS chunk (keeps PE dense while ACT/DVE of
            # chunk k-1 finish)
            if step5 and prev is not None:
                pk_x, pk = prev
                pb = pk // CPB
                if pk % CPB == 0:
                    a_ps = apool.tile([1, TD], f32, tag="aps")
                first = pk % CPB == 0
                last = pk % CPB == CPB - 1
                nc.tensor.matmul(
                    a_ps[0:1, 0:H], e_cols[:, pk : pk + 1], pk_x[:, 0:H],
                    start=first, stop=False, skip_group_check=True,
                )
                nc.tensor.matmul(
                    a_ps[0:1, H:TD], e_cols[:, pk : pk + 1], pk_x[:, H:TD],
                    start=first, stop=last, skip_group_check=True,
                )
                if last:
                    nc.vector.tensor_copy(out=a_sb[0:1, pb, :], in_=a_ps)

            if sact:
                # bias add (DVE, per half) + tanh (ACT) -> poids
                poids = ppool.tile([P, TD], f32, tag="poids")
                if qb_dve:
                    pre = prepool.tile([P, TD], f32, tag="pre")
                    nc.vector.tensor_add(
                        out=pre[:, 0:H], in0=ps0, in1=qb_bc[:, 0:H]
                    )
                    nc.vector.tensor_add(
                        out=pre[:, H:TD], in0=ps1, in1=qb_bc[:, H:TD]
                    )
                    nc.scalar.activation(out=poids, in_=pre, func=ACT.Tanh)
                else:
                    nc.scalar.activation(out=poids[:, 0:H], in_=ps0, func=ACT.Tanh)
                    nc.scalar.activation(out=poids[:, H:TD], in_=ps1, func=ACT.Tanh)

                # s = sum_d poids * v
                s_col = scpool.tile([P, 1], f32, tag="scol")
                if use_ttr:
                    scr = spool.tile([P, TD], f32, tag="scr")
                    nc.vector.tensor_tensor_reduce(
                        out=scr, in0=poids, in1=v_bc, scale=1.0, scalar=0.0,
                        op0=mybir.AluOpType.mult, op1=mybir.AluOpType.add,
                        accum_out=s_col,
                    )
                else:
                    scr = spool.tile([P, TD], f32, tag="scr")
                    nc.vector.tensor_mul(out=scr, in0=poids, in1=v_bc)
                    nc.vector.reduce_sum(
                        out=s_col, in_=scr, axis=mybir.AxisListType.X
                    )

                # e = exp(s) into its column
                nc.scalar.activation(
                    out=e_cols[:, k : k + 1], in_=s_col, func=ACT.Exp
                )
            else:
                # debug: consume ps with a plain copy so the pipeline shape holds
                poids = ppool.tile([P, TD], f32, tag="poids")
                nc.vector.tensor_copy(out=poids[:, 0:H], in_=ps0)
                nc.vector.tensor_copy(out=poids[:, H:TD], in_=ps1)

            prev = (x_nat, k)

        # flush last chunk's step5
        if step5:
            pk_x, pk = prev
            pb = pk // CPB
            nc.tensor.matmul(
                a_ps[0:1, 0:H], e_cols[:, pk : pk + 1], pk_x[:, 0:H],
                start=False, stop=False, skip_group_check=True,
            )
            nc.tensor.matmul(
                a_ps[0:1, H:TD], e_cols[:, pk : pk + 1], pk_x[:, H:TD],
                start=False, stop=True, skip_group_check=True,
            )
            nc.vector.tensor_copy(out=a_sb[0:1, pb, :], in_=a_ps)
        else:
            nc.vector.memset(a_sb, 0.0)

        # --- epilogue ---
        # e_cols [128, 64] -> rows [64, 128] (chunk-major == (b, l) row-major)
        aw_sb = singles.tile([NCH, P], f32)
        if sact:
            tp_aw = tppool.tile([P, P], trdt, tag="tp")
            nc.tensor.transpose(
                tp_aw[0:NCH, :], e_cols if tr_f32r else as_f32(e_cols), ident
            )
            nc.vector.tensor_copy(
                out=aw_sb,
                in_=tp_aw[0:NCH, :].bitcast(f32) if tr_f32r else tp_aw[0:NCH, :],
            )
        else:
            nc.vector.memset(aw_sb, 0.0)

        if epilogue:
            # row sums -> Z
            rs = singles.tile([NCH, 1], f32)
            nc.vector.reduce_sum(out=rs, in_=aw_sb, axis=mybir.AxisListType.X)
            tp_z = tppool.tile([1, NCH], f32, tag="tp")
            id_f32 = ident[0:NCH, 0:NCH].bitcast(f32) if tr_f32r else ident[0:NCH, 0:NCH]
            nc.tensor.transpose(tp_z, rs, id_f32)
            zrow = singles.tile([1, NCH], f32)
            nc.vector.tensor_copy(out=zrow, in_=tp_z)
            zb = singles.tile([1, BSH], f32)
            nc.vector.reduce_sum(
                out=zb, in_=zrow.rearrange("p (b x) -> p b x", b=BSH),
                axis=mybir.AxisListType.X,
            )
            rz = singles.tile([1, BSH], f32)
            nc.vector.reciprocal(out=rz, in_=zb)

            # attn = A / Z  (per-b scalar on partition 0)
            for b in range(BSH):
                nc.vector.tensor_scalar_mul(
                    out=a_sb[0:1, b, :], in0=a_sb[0:1, b, :],
                    scalar1=rz[0:1, b : b + 1],
                )

            # aw = e / Z: broadcast 1/Z to a [64,1] column via a DRAM bounce
            rz_dram = nc.dram_tensor("rz_scratch", [BSH], f32)
            nc.sync.dma_start(out=rz_dram[:].unsqueeze(0), in_=rz)
            rz_col = singles.tile([NCH, 1], f32)
            for b in range(BSH):
                nc.sync.dma_start(
                    out=rz_col[b * CPB : (b + 1) * CPB, 0:1],
                    in_=rz_dram[b : b + 1].unsqueeze(-1).to_broadcast([CPB, 1]),
                )
            nc.vector.tensor_scalar_mul(out=aw_sb, in0=aw_sb, scalar1=rz_col)

        nc.sync.dma_start(out=attn_d[:, :].unsqueeze(0), in_=a_sb)
        nc.sync.dma_start(
            out=aw_d.rearrange("b (x n) -> (b x) n", n=P), in_=aw_sb
        )

    # run the bacc passes (wait splitting, reg alloc) before serialization --
    # the axon/pjrt path serializes nc as-is without calling finalize()
    nc.finalize()
    return nc


def _get_nc(**kw):
    key = tuple(sorted(kw.items()))
    if key not in _cache:
        _cache[key] = _build(**kw)
    return _cache[key]


def _prep_inputs(query, targets, W1, b1, v):
    query = np.asarray(query, dtype=np.float32)
    targets = np.asarray(targets, dtype=np.float32)
    W1 = np.asarray(W1, dtype=np.float32)
    b1 = np.asarray(b1, dtype=np.float32)
    v = np.asarray(v, dtype=np.float32)

    qb = query[0] @ W1[:, :QD].T + b1          # (B, TD)
    w_t = np.ascontiguousarray(W1[:, QD:].T)   # (c, d)
    v2 = np.ascontiguousarray(v.reshape(1, TD))

    in_maps = []
    for core in range(NCORES):
        bs = slice(core * BSH, (core + 1) * BSH)
        in_maps.append(
            {
                "targets": np.ascontiguousarray(targets[:, bs, :]),
                "w": w_t,
                "qb": np.ascontiguousarray(qb[bs]),
                "v": v2,
                "ident": np.eye(P, dtype=np.float32),
                "ones": np.ones((1, P), dtype=np.float32),
            }
        )
    return in_maps


def _assemble(results):
    attn = np.concatenate([r["attn"] for r in results], axis=0)[None]   # (1,B,TD)
    aw = np.concatenate([r["aw"] for r in results], axis=0)[:, None, :]  # (B,1,L)
    return attn.astype(np.float32), aw.astype(np.float32)


def run(query, targets, W1, b1, v, trace=False, **build_kw):
    from concourse.bass_utils import run_bass_kernel_spmd

    nc = _get_nc(**build_kw)
    in_maps = _prep_inputs(query, targets, W1, b1, v)
    res = run_bass_kernel_spmd(nc, in_maps, core_ids=list(range(NCORES)), trace=trace)
    return _assemble(res.results), res


def kernel(query, targets, W1, b1, v):
    (attn, aw), _ = run(query, targets, W1, b1, v, trace=False)
    return attn, aw


# revision 42
# speedup vs baseline: 1.0161x; 1.0161x over previous
# Additive attention kernel for Trainium2, SPMD over 8 NeuronCores.
#
# Reference computation (L=2048, B=32, Qd=Td=1024):
#   pre[l,b,d]  = sum_c T[l,b,c]*W1t[d,c] + (q[b] @ W1q.T + b1)[d]
#   poids       = tanh(pre)
#   s[l,b]      = sum_d poids[l,b,d] * v[d]
#   aw          = softmax_l(s)
#   attn[b,d]   = sum_l aw[l,b] * T[l,b,d]
#
# Sharding: data-parallel over batch. Each core gets 4 of 32 batch elements
# (full L), so softmax over L and the weighted sum need no collectives.
#
# Per-core dataflow (64 chunks of 128 (b,l)-rows, b-major):
#   DMA T tile [128n, 1024c] -> PE transpose -> X_T [128c, 8, 128n]
#   main matmul (fp32r): psum[n,d] += X_T(c)^T @ W1tT(c,d), 8 k-steps x 2 halves
#   + rank-1 matmul ones[1,128] (x) qb[b] adds the query/bias term
#   ACT tanh -> poids; DVE tensor_tensor_reduce(poids * v_bc) -> s column
#   ACT exp(s) -> e column (softmax without max subtraction: |s| <= sum|v| ~ 16,
#   exp stays well inside fp32 range; aw = e/Z is mathematically identical)
#   step5 matmul: A[b] += e_col^T @ T_tile, PSUM-accumulated over the 16
#   chunks of each batch stripe.
# End: PE-transpose e-columns to rows, row-sums -> Z, normalize aw and A on
# device, DMA out.

import numpy as np

L, B, QD, TD = 2048, 32, 1024, 1024
NCORES = 8
BSH = B // NCORES          # 4 batch elements per core
P = 128                    # partitions
NCH = (L * BSH) // P       # 64 chunks per core
CPB = L // P               # 16 chunks per batch element
KC = TD // P               # 8 contraction sub-blocks
H = TD // 2                # 512, half of the free dim (one PSUM bank)

_cache = {}


def _build(mm_f32r=True, tr_f32r=True, step5=True, epilogue=True, sact=True,
           use_ttr=False, qb_dve=True):
    # use_ttr=True (fused DVE tensor_tensor_reduce) crashes the exec unit on
    # this runtime (NRT_EXEC_UNIT_UNRECOVERABLE); the mul+reduce pair works.
    from contextlib import ExitStack

    import concourse.bass as bass
    import concourse.bacc as bacc_mod
    import concourse.tile as tile
    import concourse.mybir as mybir

    f32 = mybir.dt.float32
    f32r = mybir.dt.float32r
    ACT = mybir.ActivationFunctionType

    # dtype for tensors feeding the fp32r matmuls (must be materialized as
    # float32r for the BIR verifier; bit layout is identical to fp32)
    mmdt = f32r if mm_f32r else f32

    def dram_mm(ap):
        # DRAM-side AP view for DMA into an mmdt tile (exact bits)
        return ap.bitcast(f32r) if mm_f32r else ap

    def as_f32(ap):
        # view an mmdt tile as plain fp32 (for non-f32r consumers)
        return ap.bitcast(f32) if mm_f32r else ap

    nc = bacc_mod.Bacc("TRN2")

    t_d = nc.declare_dram_parameter("targets", [L, BSH, TD], f32, isOutput=False)
    w_d = nc.declare_dram_parameter("w", [TD, TD], f32, isOutput=False)  # W1t.T [c,d]
    qb_d = nc.declare_dram_parameter("qb", [BSH, TD], f32, isOutput=False)
    v_d = nc.declare_dram_parameter("v", [1, TD], f32, isOutput=False)
    id_d = nc.declare_dram_parameter("ident", [P, P], f32, isOutput=False)
    ones_d = nc.declare_dram_parameter("ones", [1, P], f32, isOutput=False)
    attn_d = nc.declare_dram_parameter("attn", [BSH, TD], f32, isOutput=True)
    aw_d = nc.declare_dram_parameter("aw", [BSH, L], f32, isOutput=True)

    with tile.TileContext(nc) as tc, ExitStack() as ctx:
        singles = ctx.enter_context(tc.tile_pool(name="singles", bufs=1))
        xpool = ctx.enter_context(tc.tile_pool(name="xnat", bufs=3))
        xtpool = ctx.enter_context(tc.tile_pool(name="xt", bufs=3))
        ppool = ctx.enter_context(tc.tile_pool(name="poids", bufs=3))
        spool = ctx.enter_context(tc.tile_pool(name="scratch", bufs=2))
        scpool = ctx.enter_context(tc.tile_pool(name="scol", bufs=3))
        prepool = ctx.enter_context(tc.tile_pool(name="pre", bufs=3))
        qbcpool = ctx.enter_context(tc.tile_pool(name="qbc", bufs=2))
        tppool = ctx.enter_context(tc.tile_pool(name="tpsum", bufs=2, space="PSUM"))
        mmpool = ctx.enter_context(tc.tile_pool(name="mmpsum", bufs=2, space="PSUM"))
        apool = ctx.enter_context(tc.tile_pool(name="apsum", bufs=1, space="PSUM"))

        # --- resident tiles ---
        # small setup tensors first on HWDGE so the first transposes/matmuls
        # aren't queued behind the 4MB W transfer; W itself goes over SWDGE
        # (gpsimd) so the HWDGE queues stay dedicated to the x_nat stream
        trdt = mmdt if tr_f32r else f32
        ident = singles.tile([P, P], trdt)
        nc.sync.dma_start(
            out=ident, in_=id_d[:, :].bitcast(trdt) if tr_f32r else id_d[:, :]
        )

        v_bc = singles.tile([P, TD], f32)
        nc.sync.dma_start(out=v_bc, in_=v_d[0:1, :].to_broadcast([P, TD]))

        if not qb_dve:
            qb_sb = singles.tile([1, BSH, TD], mmdt)
            nc.sync.dma_start(out=qb_sb, in_=dram_mm(qb_d[:, :].unsqueeze(0)))
            ones_sb = singles.tile([1, P], mmdt)
            nc.sync.dma_start(out=ones_sb, in_=dram_mm(ones_d[:, :]))

        w_sb = singles.tile([P, KC, TD], mmdt)
        nc.gpsimd.dma_start(
            out=w_sb, in_=dram_mm(w_d.rearrange("(ci cp) d -> cp ci d", cp=P))
        )

        e_cols = singles.tile([P, NCH], mmdt)   # exp(s), one column per chunk
        a_sb = singles.tile([1, BSH, TD], f32)  # unnormalized attn rows

        # --- main loop ---
        # state carried between iterations for the software-pipelined step5
        prev = None  # (x_nat, k) of previous chunk
        a_ps = None
        qb_bc = None

        for k in range(NCH):
            b = k // CPB
            l0 = (k % CPB) * P

            if qb_dve and k % CPB == 0:
                # per-stripe broadcast of qb[b] across partitions (DVE adds it
                # to the matmul output instead of a PE rank-1 matmul)
                qb_bc = qbcpool.tile([P, TD], f32, tag="qbc")
                nc.sync.dma_start(
                    out=qb_bc, in_=qb_d[b : b + 1, :].to_broadcast([P, TD])
                )

            x_nat = xpool.tile([P, TD], mmdt, tag="xnat")
            nc.sync.dma_start(out=x_nat, in_=dram_mm(t_d[l0 : l0 + P, b, :]))

            # transpose x_nat -> x_t [c, ci, n] (plain fp32 transpose path).
            # Two separate tiles, one per copy engine, so each matmul has a
            # single producer to wait on (walrus limits sync waits per inst).
            x_t_lo = xtpool.tile([P, 4, P], mmdt, tag="xtlo")
            x_t_hi = xtpool.tile([P, 4, P], mmdt, tag="xthi")
            for h in range(2):
                tp = tppool.tile([P, 4, P], trdt, tag="tp")
                for j in range(4):
                    ci = 4 * h + j
                    src = x_nat[:, ci * P : (ci + 1) * P]
                    nc.tensor.transpose(
                        tp[:, j, :], src if tr_f32r else as_f32(src), ident
                    )
                if h == 0:
                    nc.vector.tensor_copy(out=x_t_lo, in_=tp)
                else:
                    nc.scalar.copy(out=x_t_hi, in_=tp)

            # main matmul: psum[n, d] over 2 halves, accumulate over KC blocks
            ps = mmpool.tile([P, TD], f32, tag="mm")
            ps0 = ps[:, 0:H]
            ps1 = ps[:, H:TD]
            for ci in range(KC):
                lhsT = x_t_lo[:, ci, :] if ci < 4 else x_t_hi[:, ci - 4, :]
                last_ci = qb_dve and ci == KC - 1
                nc.tensor.matmul(
                    ps0, lhsT, w_sb[:, ci, 0:H],
                    start=(ci == 0), stop=last_ci,
                )
                nc.tensor.matmul(
                    ps1, lhsT, w_sb[:, ci, H:TD],
                    start=(ci == 0), stop=last_ci,
                )
            if not qb_dve:
                # rank-1 bias: ones (x) qb[b]
                nc.tensor.matmul(
                    ps0, ones_sb, qb_sb[0:1, b, 0:H],
                    start=False, stop=True,
                )
                nc.tensor.matmul(
                    ps1, ones_sb, qb_sb[0:1, b, H:TD],
                    start=False, stop=True,
                )

            # step5 for the PREVIOUS chunk (keeps PE dense while ACT/DVE of
            # chunk k-1 finish)
            if step5 and prev is not None:
                pk_x, pk = prev
                pb = pk // CPB
                if pk % CPB == 0:
                    a_ps = apool.tile([1, TD], f32, tag="aps")
                first = pk % CPB == 0
                last = pk % CPB == CPB - 1
                nc.tensor.matmul(
                    a_ps[0:1, 0:H], e_cols[:, pk : pk + 1], pk_x[:, 0:H],
                    start=first, stop=False, skip_group_check=True,
                )
                nc.tensor.matmul(
                    a_ps[0:1, H:TD], e_cols[:, pk : pk + 1], pk_x[:, H:TD],
                    start=first, stop=last, skip_group_check=True,
                )
                if last:
                    nc.vector.tensor_copy(out=a_sb[0:1, pb, :], in_=a_ps)

            if sact:
                # bias add (DVE, per half) + tanh (ACT) -> poids
                poids = ppool.tile([P, TD], f32, tag="poids")
                if qb_dve:
                    pre = prepool.tile([P, TD], f32, tag="pre")
                    nc.vector.tensor_add(out=pre, in0=ps, in1=qb_bc)
                    nc.scalar.activation(out=poids, in_=pre, func=ACT.Tanh)
                else:
                    nc.scalar.activation(out=poids, in_=ps, func=ACT.Tanh)

                # s = sum_d poids * v
                s_col = scpool.tile([P, 1], f32, tag="scol")
                if use_ttr:
                    scr = spool.tile([P, TD], f32, tag="scr")
                    nc.vector.tensor_tensor_reduce(
                        out=scr, in0=poids, in1=v_bc, scale=1.0, scalar=0.0,
                        op0=mybir.AluOpType.mult, op1=mybir.AluOpType.add,
                        accum_out=s_col,
                    )
                else:
                    scr = spool.tile([P, TD], f32, tag="scr")
                    nc.vector.tensor_mul(out=scr, in0=poids, in1=v_bc)
                    nc.vector.reduce_sum(
                        out=s_col, in_=scr, axis=mybir.AxisListType.X
                    )

                # e = exp(s) into its column
                nc.scalar.activation(
                    out=e_cols[:, k : k + 1], in_=s_col, func=ACT.Exp
                )
            else:
                # debug: consume ps with a plain copy so the pipeline shape holds
                poids = ppool.tile([P, TD], f32, tag="poids")
                nc.vector.tensor_copy(out=poids, in_=ps)

            prev = (x_nat, k)

        # flush last chunk's step5
        if step5:
            pk_x, pk = prev
            pb = pk // CPB
            nc.tensor.matmul(
                a_ps[0:1, 0:H], e_cols[:, pk : pk + 1], pk_x[:, 0:H],
                start=False, stop=False, skip_group_check=True,
            )
            nc.tensor.matmul(
                a_ps[0:1, H:TD], e_cols[:, pk : pk + 1], pk_x[:, H:TD],
                start=False, stop=True, skip_group_check=True,
            )
            nc.vector.tensor_copy(out=a_sb[0:1, pb, :], in_=a_ps)
        else:
            nc.vector.memset(a_sb, 0.0)

        # --- epilogue ---
        # e_cols [128, 64] -> rows [64, 128] (chunk-major == (b, l) row-major)
        aw_sb = singles.tile([NCH, P], f32)
        if sact:
            tp_aw = tppool.tile([P, P], trdt, tag="tp")
            nc.tensor.transpose(
                tp_aw[0:NCH, :], e_cols if tr_f32r else as_f32(e_cols), ident
            )
            nc.vector.tensor_copy(
                out=aw_sb,
                in_=tp_aw[0:NCH, :].bitcast(f32) if tr_f32r else tp_aw[0:NCH, :],
            )
        else:
            nc.vector.memset(aw_sb, 0.0)

        if epilogue:
            # row sums -> Z
            rs = singles.tile([NCH, 1], f32)
            nc.vector.reduce_sum(out=rs, in_=aw_sb, axis=mybir.AxisListType.X)
            tp_z = tppool.tile([1, NCH], f32, tag="tp")
            id_f32 = ident[0:NCH, 0:NCH].bitcast(f32) if tr_f32r else ident[0:NCH, 0:NCH]
            nc.tensor.transpose(tp_z, rs, id_f32)
            zrow = singles.tile([1, NCH], f32)
            nc.vector.tensor_copy(out=zrow, in_=tp_z)
            zb = singles.tile([1, BSH], f32)
            nc.vector.reduce_sum(
                out=zb, in_=zrow.rearrange("p (b x) -> p b x", b=BSH),
                axis=mybir.AxisListType.X,
            )
            rz = singles.tile([1, BSH], f32)
            nc.vector.reciprocal(out=rz, in_=zb)

            # attn = A / Z  (per-b scalar on partition 0)
            for b in range(BSH):
                nc.vector.tensor_scalar_mul(
                    out=a_sb[0:1, b, :], in0=a_sb[0:1, b, :],
                    scalar1=rz[0:1, b : b + 1],
                )

            # aw = e / Z: broadcast 1/Z to a [64,1] column via a DRAM bounce
            rz_dram = nc.dram_tensor("rz_scratch", [BSH], f32)
            nc.sync.dma_start(out=rz_dram[:].unsqueeze(0), in_=rz)
            rz_col = singles.tile([NCH, 1], f32)
            for b in range(BSH):
                nc.sync.dma_start(
                    out=rz_col[b * CPB : (b + 1) * CPB, 0:1],
                    in_=rz_dram[b : b + 1].unsqueeze(-1).to_broadcast([CPB, 1]),
                )
            nc.vector.tensor_scalar_mul(out=aw_sb, in0=aw_sb, scalar1=rz_col)

        nc.sync.dma_start(out=attn_d[:, :].unsqueeze(0), in_=a_sb)
        nc.sync.dma_start(
            out=aw_d.rearrange("b (x n) -> (b x) n", n=P), in_=aw_sb
        )

    # run the bacc passes (wait splitting, reg alloc) before serialization --
    # the axon/pjrt path serializes nc as-is without calling finalize()
    nc.finalize()
    return nc


def _get_nc(**kw):
    key = tuple(sorted(kw.items()))
    if key not in _cache:
        _cache[key] = _build(**kw)
    return _cache[key]


def _prep_inputs(query, targets, W1, b1, v):
    query = np.asarray(query, dtype=np.float32)
    targets = np.asarray(targets, dtype=np.float32)
    W1 = np.asarray(W1, dtype=np.float32)
    b1 = np.asarray(b1, dtype=np.float32)
    v = np.asarray(v, dtype=np.float32)

    qb = query[0] @ W1[:, :QD].T + b1          # (B, TD)
    w_t = np.ascontiguousarray(W1[:, QD:].T)   # (c, d)
    v2 = np.ascontiguousarray(v.reshape(1, TD))

    in_maps = []
    for core in range(NCORES):
        bs = slice(core * BSH, (core + 1) * BSH)
        in_maps.append(
            {
                "targets": np.ascontiguousarray(targets[:, bs, :]),
                "w": w_t,
                "qb": np.ascontiguousarray(qb[bs]),
                "v": v2,
                "ident": np.eye(P, dtype=np.float32),
                "ones": np.ones((1, P), dtype=np.float32),
            }
        )
    return in_maps


def _assemble(results):
    attn = np.concatenate([r["attn"] for r in results], axis=0)[None]   # (1,B,TD)
    aw = np.concatenate([r["aw"] for r in results], axis=0)[:, None, :]  # (B,1,L)
    return attn.astype(np.float32), aw.astype(np.float32)


def run(query, targets, W1, b1, v, trace=False, **build_kw):
    from concourse.bass_utils import run_bass_kernel_spmd

    nc = _get_nc(**build_kw)
    in_maps = _prep_inputs(query, targets, W1, b1, v)
    res = run_bass_kernel_spmd(nc, in_maps, core_ids=list(range(NCORES)), trace=trace)
    return _assemble(res.results), res


def kernel(query, targets, W1, b1, v):
    (attn, aw), _ = run(query, targets, W1, b1, v, trace=False)
    return attn, aw


# revision 43
# speedup vs baseline: 1.1822x; 1.1635x over previous
# Additive attention kernel for Trainium2, SPMD over 8 NeuronCores.
#
# Reference computation (L=2048, B=32, Qd=Td=1024):
#   pre[l,b,d]  = sum_c T[l,b,c]*W1t[d,c] + (q[b] @ W1q.T + b1)[d]
#   poids       = tanh(pre)
#   s[l,b]      = sum_d poids[l,b,d] * v[d]
#   aw          = softmax_l(s)
#   attn[b,d]   = sum_l aw[l,b] * T[l,b,d]
#
# Sharding: data-parallel over batch. Each core gets 4 of 32 batch elements
# (full L), so softmax over L and the weighted sum need no collectives.
#
# Per-core dataflow (64 chunks of 128 (b,l)-rows, b-major):
#   DMA T tile [128n, 1024c] -> PE transpose -> X_T [128c, 8, 128n]
#   main matmul (fp32r): psum[n,d] += X_T(c)^T @ W1tT(c,d), 8 k-steps x 2 halves
#   + rank-1 matmul ones[1,128] (x) qb[b] adds the query/bias term
#   ACT tanh -> poids; DVE tensor_tensor_reduce(poids * v_bc) -> s column
#   ACT exp(s) -> e column (softmax without max subtraction: |s| <= sum|v| ~ 16,
#   exp stays well inside fp32 range; aw = e/Z is mathematically identical)
#   step5 matmul: A[b] += e_col^T @ T_tile, PSUM-accumulated over the 16
#   chunks of each batch stripe.
# End: PE-transpose e-columns to rows, row-sums -> Z, normalize aw and A on
# device, DMA out.

import numpy as np

L, B, QD, TD = 2048, 32, 1024, 1024
NCORES = 8
BSH = B // NCORES          # 4 batch elements per core
P = 128                    # partitions
NCH = (L * BSH) // P       # 64 chunks per core
CPB = L // P               # 16 chunks per batch element
KC = TD // P               # 8 contraction sub-blocks
H = TD // 2                # 512, half of the free dim (one PSUM bank)

_cache = {}


def _build(mm_f32r=True, tr_f32r=True, step5=True, epilogue=True, sact=True,
           use_ttr=False, qb_dve=True):
    # use_ttr=True (fused DVE tensor_tensor_reduce) crashes the exec unit on
    # this runtime (NRT_EXEC_UNIT_UNRECOVERABLE); the mul+reduce pair works.
    from contextlib import ExitStack

    import concourse.bass as bass
    import concourse.bacc as bacc_mod
    import concourse.tile as tile
    import concourse.mybir as mybir

    f32 = mybir.dt.float32
    f32r = mybir.dt.float32r
    ACT = mybir.ActivationFunctionType

    # dtype for tensors feeding the fp32r matmuls (must be materialized as
    # float32r for the BIR verifier; bit layout is identical to fp32)
    mmdt = f32r if mm_f32r else f32

    def dram_mm(ap):
        # DRAM-side AP view for DMA into an mmdt tile (exact bits)
        return ap.bitcast(f32r) if mm_f32r else ap

    def as_f32(ap):
        # view an mmdt tile as plain fp32 (for non-f32r consumers)
        return ap.bitcast(f32) if mm_f32r else ap

    nc = bacc_mod.Bacc("TRN2")

    t_d = nc.declare_dram_parameter("targets", [L, BSH, TD], f32, isOutput=False)
    w_d = nc.declare_dram_parameter("w", [TD, TD], f32, isOutput=False)  # W1t.T [c,d]
    qb_d = nc.declare_dram_parameter("qb", [BSH, TD], f32, isOutput=False)
    v_d = nc.declare_dram_parameter("v", [1, TD], f32, isOutput=False)
    id_d = nc.declare_dram_parameter("ident", [P, P], f32, isOutput=False)
    ones_d = nc.declare_dram_parameter("ones", [1, P], f32, isOutput=False)
    attn_d = nc.declare_dram_parameter("attn", [BSH, TD], f32, isOutput=True)
    aw_d = nc.declare_dram_parameter("aw", [BSH, L], f32, isOutput=True)

    with tile.TileContext(nc) as tc, ExitStack() as ctx:
        singles = ctx.enter_context(tc.tile_pool(name="singles", bufs=1))
        xpool = ctx.enter_context(tc.tile_pool(name="xnat", bufs=5))
        xtpool = ctx.enter_context(tc.tile_pool(name="xt", bufs=3))
        ppool = ctx.enter_context(tc.tile_pool(name="poids", bufs=3))
        spool = ctx.enter_context(tc.tile_pool(name="scratch", bufs=2))
        scpool = ctx.enter_context(tc.tile_pool(name="scol", bufs=3))
        prepool = ctx.enter_context(tc.tile_pool(name="pre", bufs=3))
        qbcpool = ctx.enter_context(tc.tile_pool(name="qbc", bufs=2))
        tppool = ctx.enter_context(tc.tile_pool(name="tpsum", bufs=2, space="PSUM"))
        mmpool = ctx.enter_context(tc.tile_pool(name="mmpsum", bufs=2, space="PSUM"))
        apool = ctx.enter_context(tc.tile_pool(name="apsum", bufs=1, space="PSUM"))

        # --- resident tiles ---
        # small setup tensors first on HWDGE so the first transposes/matmuls
        # aren't queued behind the 4MB W transfer; W itself goes over SWDGE
        # (gpsimd) so the HWDGE queues stay dedicated to the x_nat stream
        trdt = mmdt if tr_f32r else f32
        ident = singles.tile([P, P], trdt)
        nc.sync.dma_start(
            out=ident, in_=id_d[:, :].bitcast(trdt) if tr_f32r else id_d[:, :]
        )

        v_bc = singles.tile([P, TD], f32)
        nc.sync.dma_start(out=v_bc, in_=v_d[0:1, :].to_broadcast([P, TD]))

        if not qb_dve:
            qb_sb = singles.tile([1, BSH, TD], mmdt)
            nc.sync.dma_start(out=qb_sb, in_=dram_mm(qb_d[:, :].unsqueeze(0)))
            ones_sb = singles.tile([1, P], mmdt)
            nc.sync.dma_start(out=ones_sb, in_=dram_mm(ones_d[:, :]))

        w_sb = singles.tile([P, KC, TD], mmdt)
        nc.gpsimd.dma_start(
            out=w_sb, in_=dram_mm(w_d.rearrange("(ci cp) d -> cp ci d", cp=P))
        )

        e_cols = singles.tile([P, NCH], mmdt)   # exp(s), one column per chunk
        a_sb = singles.tile([1, BSH, TD], f32)  # unnormalized attn rows

        # --- main loop ---
        # Deep software pipeline so every engine's FIFO only holds input-ready
        # ops (cross-engine chains otherwise serialize chunk k+1's x_t copy
        # behind chunk k's add/tanh/mul/reduce):
        #   iter k emits: transposes/copies/mains(k), step5(k-3),
        #                 add+tanh(k-1), mul/reduce/exp(k-2)
        recs = {}
        a_ps = None
        qb_bc = None

        def emit_step5(pk):
            nonlocal a_ps
            r = recs[pk]
            pb = pk // CPB
            if pk % CPB == 0:
                a_ps = apool.tile([1, TD], f32, tag="aps")
            first = pk % CPB == 0
            last = pk % CPB == CPB - 1
            nc.tensor.matmul(
                a_ps[0:1, 0:H], e_cols[:, pk : pk + 1], r["x"][:, 0:H],
                start=first, stop=False, skip_group_check=True,
            )
            nc.tensor.matmul(
                a_ps[0:1, H:TD], e_cols[:, pk : pk + 1], r["x"][:, H:TD],
                start=first, stop=last, skip_group_check=True,
            )
            if last:
                nc.vector.tensor_copy(out=a_sb[0:1, pb, :], in_=a_ps)

        def emit_add_tanh(pk):
            r = recs[pk]
            pre = prepool.tile([P, TD], f32, tag="pre")
            nc.vector.tensor_add(out=pre, in0=r["ps"], in1=r["qb"])
            poids = ppool.tile([P, TD], f32, tag="poids")
            nc.scalar.activation(out=poids, in_=pre, func=ACT.Tanh)
            r["poids"] = poids

        def emit_score(pk):
            r = recs[pk]
            scr = spool.tile([P, TD], f32, tag="scr")
            s_col = scpool.tile([P, 1], f32, tag="scol")
            nc.vector.tensor_mul(out=scr, in0=r["poids"], in1=v_bc)
            nc.vector.reduce_sum(out=s_col, in_=scr, axis=mybir.AxisListType.X)
            nc.scalar.activation(
                out=e_cols[:, pk : pk + 1], in_=s_col, func=ACT.Exp
            )

        for k in range(NCH):
            b = k // CPB
            l0 = (k % CPB) * P

            if qb_dve and k % CPB == 0:
                qb_bc = qbcpool.tile([P, TD], f32, tag="qbc")
                nc.sync.dma_start(
                    out=qb_bc, in_=qb_d[b : b + 1, :].to_broadcast([P, TD])
                )

            x_nat = xpool.tile([P, TD], mmdt, tag="xnat")
            nc.sync.dma_start(out=x_nat, in_=dram_mm(t_d[l0 : l0 + P, b, :]))

            x_t_lo = xtpool.tile([P, 4, P], mmdt, tag="xtlo")
            x_t_hi = xtpool.tile([P, 4, P], mmdt, tag="xthi")
            for h in range(2):
                tp = tppool.tile([P, 4, P], trdt, tag="tp")
                for j in range(4):
                    ci = 4 * h + j
                    src = x_nat[:, ci * P : (ci + 1) * P]
                    nc.tensor.transpose(
                        tp[:, j, :], src if tr_f32r else as_f32(src), ident
                    )
                if h == 0:
                    nc.vector.tensor_copy(out=x_t_lo, in_=tp)
                else:
                    nc.scalar.copy(out=x_t_hi, in_=tp)

            # fill PE while the copies land
            if sact and step5 and k >= 3:
                emit_step5(k - 3)

            ps = mmpool.tile([P, TD], f32, tag="mm")
            for ci in range(KC):
                lhsT = x_t_lo[:, ci, :] if ci < 4 else x_t_hi[:, ci - 4, :]
                nc.tensor.matmul(
                    ps[:, 0:H], lhsT, w_sb[:, ci, 0:H],
                    start=(ci == 0), stop=(ci == KC - 1),
                )
                nc.tensor.matmul(
                    ps[:, H:TD], lhsT, w_sb[:, ci, H:TD],
                    start=(ci == 0), stop=(ci == KC - 1),
                )
            recs[k] = {"x": x_nat, "ps": ps, "qb": qb_bc}

            if sact and k >= 1:
                emit_add_tanh(k - 1)
            if sact and k >= 2:
                emit_score(k - 2)

        # drain the pipeline
        if sact:
            emit_add_tanh(NCH - 1)
            emit_score(NCH - 2)
            emit_score(NCH - 1)
            if step5:
                for pk in (NCH - 3, NCH - 2, NCH - 1):
                    emit_step5(pk)

        # --- epilogue ---
        # e_cols [128, 64] -> rows [64, 128] (chunk-major == (b, l) row-major)
        aw_sb = singles.tile([NCH, P], f32)
        if sact:
            tp_aw = tppool.tile([P, P], trdt, tag="tp")
            nc.tensor.transpose(
                tp_aw[0:NCH, :], e_cols if tr_f32r else as_f32(e_cols), ident
            )
            nc.vector.tensor_copy(
                out=aw_sb,
                in_=tp_aw[0:NCH, :].bitcast(f32) if tr_f32r else tp_aw[0:NCH, :],
            )
        else:
            nc.vector.memset(aw_sb, 0.0)

        if epilogue:
            # row sums -> Z
            rs = singles.tile([NCH, 1], f32)
            nc.vector.reduce_sum(out=rs, in_=aw_sb, axis=mybir.AxisListType.X)
            tp_z = tppool.tile([1, NCH], f32, tag="tp")
            id_f32 = ident[0:NCH, 0:NCH].bitcast(f32) if tr_f32r else ident[0:NCH, 0:NCH]
            nc.tensor.transpose(tp_z, rs, id_f32)
            zrow = singles.tile([1, NCH], f32)
            nc.vector.tensor_copy(out=zrow, in_=tp_z)
            zb = singles.tile([1, BSH], f32)
            nc.vector.reduce_sum(
                out=zb, in_=zrow.rearrange("p (b x) -> p b x", b=BSH),
                axis=mybir.AxisListType.X,
            )
            rz = singles.tile([1, BSH], f32)
            nc.vector.reciprocal(out=rz, in_=zb)

            # attn = A / Z  (per-b scalar on partition 0)
            for b in range(BSH):
                nc.vector.tensor_scalar_mul(
                    out=a_sb[0:1, b, :], in0=a_sb[0:1, b, :],
                    scalar1=rz[0:1, b : b + 1],
                )

            # aw = e / Z: broadcast 1/Z to a [64,1] column via a DRAM bounce
            rz_dram = nc.dram_tensor("rz_scratch", [BSH], f32)
            nc.sync.dma_start(out=rz_dram[:].unsqueeze(0), in_=rz)
            rz_col = singles.tile([NCH, 1], f32)
            for b in range(BSH):
                nc.sync.dma_start(
                    out=rz_col[b * CPB : (b + 1) * CPB, 0:1],
                    in_=rz_dram[b : b + 1].unsqueeze(-1).to_broadcast([CPB, 1]),
                )
            nc.vector.tensor_scalar_mul(out=aw_sb, in0=aw_sb, scalar1=rz_col)

        nc.sync.dma_start(out=attn_d[:, :].unsqueeze(0), in_=a_sb)
        nc.sync.dma_start(
            out=aw_d.rearrange("b (x n) -> (b x) n", n=P), in_=aw_sb
        )

    # run the bacc passes (wait splitting, reg alloc) before serialization --
    # the axon/pjrt path serializes nc as-is without calling finalize()
    nc.finalize()
    return nc


def _get_nc(**kw):
    key = tuple(sorted(kw.items()))
    if key not in _cache:
        _cache[key] = _build(**kw)
    return _cache[key]


def _prep_inputs(query, targets, W1, b1, v):
    query = np.asarray(query, dtype=np.float32)
    targets = np.asarray(targets, dtype=np.float32)
    W1 = np.asarray(W1, dtype=np.float32)
    b1 = np.asarray(b1, dtype=np.float32)
    v = np.asarray(v, dtype=np.float32)

    qb = query[0] @ W1[:, :QD].T + b1          # (B, TD)
    w_t = np.ascontiguousarray(W1[:, QD:].T)   # (c, d)
    v2 = np.ascontiguousarray(v.reshape(1, TD))

    in_maps = []
    for core in range(NCORES):
        bs = slice(core * BSH, (core + 1) * BSH)
        in_maps.append(
            {
                "targets": np.ascontiguousarray(targets[:, bs, :]),
                "w": w_t,
                "qb": np.ascontiguousarray(qb[bs]),
                "v": v2,
                "ident": np.eye(P, dtype=np.float32),
                "ones": np.ones((1, P), dtype=np.float32),
            }
        )
    return in_maps


def _assemble(results):
    attn = np.concatenate([r["attn"] for r in results], axis=0)[None]   # (1,B,TD)
    aw = np.concatenate([r["aw"] for r in results], axis=0)[:, None, :]  # (B,1,L)
    return attn.astype(np.float32), aw.astype(np.float32)


def run(query, targets, W1, b1, v, trace=False, **build_kw):
    from concourse.bass_utils import run_bass_kernel_spmd

    nc = _get_nc(**build_kw)
    in_maps = _prep_inputs(query, targets, W1, b1, v)
    res = run_bass_kernel_spmd(nc, in_maps, core_ids=list(range(NCORES)), trace=trace)
    return _assemble(res.results), res


def kernel(query, targets, W1, b1, v):
    (attn, aw), _ = run(query, targets, W1, b1, v, trace=False)
    return attn, aw
